# revision 1
# baseline (speedup 1.0000x reference)
"""Causal multi-head self-attention on 8 TRN2 NeuronCores.

Sharding: batch (2) x head-group (4 heads = 256 contiguous features) -> 8 cores.
Each core computes q/k/v projections for its 256 output features from its
batch's full activations, then causal attention for its 4 heads. No
collectives: the host concatenates the 8 [S, 256] shards.

Kernel layout choices:
  - all matmuls run as float32r (full-rate fp32 PE mode, ~1 cyc/row at
    N>=256; tiles feeding matmuls are declared float32r so producers round).
  - qT/kT stored [d, s] (head dim on partitions) so scores come out
    transposed [k, q]; PV then consumes exp(scores) directly as the moving
    operand -- no probs transpose. Row sums come from a ones-column appended
    to V (65-wide PV stationary); normalization happens after a final small
    PE transpose of z.
  - softmax skips max-subtraction (scores ~ N(0,1) after the 1/8 scale).
  - causal mask: diagonal 128x128 blocks get an additive -8e9 mask on PSUM
    before exp; fully-masked columns are never computed (column-restricted
    scores/exp/PV windows).
  - identity / mask / ones constants are DMA'd from host inputs (no gpsimd
    ucode load, shorter startup).
  - the whole schedule is emitted as generators explicitly interleaved in
    program order (engine queues are in-order): weight prep with s-group 0,
    and attention for query group g with the transpose/projection units of
    s-group g+1 (causality makes group g data-complete after s-group g).
    Scores/exp/PV run as a batch-2 software pipeline over two head-chains.
"""

import sys

import numpy as np

sys.path.insert(0, "/opt/trn_rl_repo")

import concourse.bass as bass
import concourse.tile as tile
from concourse import bacc, mybir
from concourse.bass_utils import run_bass_kernel_spmd
from concourse.masks import make_identity

B, S, D, H, DK = 2, 2048, 1024, 16, 64
NCORES = 8
HD = 256  # output features per core (4 heads x 64)
NHC = 4  # heads per core
NST = S // 128  # 16 s-tiles
NCC = D // 128  # 8 contraction chunks
NG = S // 512  # 4 query groups of 512

f32 = mybir.dt.float32
f32r = mybir.dt.float32r
bf16 = mybir.dt.bfloat16
AF = mybir.ActivationFunctionType
PSUM = bass.MemorySpace.PSUM


def _body(nc, tc, x, wq, wk, wv, bq, bk, bv, out, cid, cmask, cones):
    with (
        tc.tile_pool(name="persist", bufs=1) as persist,
        tc.tile_pool(name="xnat", bufs=3) as x_pool,
        tc.tile_pool(name="wnat", bufs=1) as w_pool,
        tc.tile_pool(name="u", bufs=12) as u_pool,
        tc.tile_pool(name="zc", bufs=2) as zc_pool,
        tc.tile_pool(name="small", bufs=2) as small,
        tc.tile_pool(name="psum_big", bufs=6, space=PSUM) as psum_big,
        tc.tile_pool(name="psum_acc", bufs=2, space=PSUM) as psum_acc,
    ):
        ident = persist.tile([128, 128], f32)
        nc.sync.dma_start(out=ident[:], in_=cid.bitcast(f32))
        ident_r = persist.tile([128, 128], f32r)
        nc.sync.dma_start(out=ident_r[:], in_=cid)

        # dmask[k, q] = 0.0 if q >= k else -8e9 (additive causal, diagonal block)
        dmask = persist.tile([128, 128], f32)
        nc.sync.dma_start(out=dmask[:], in_=cmask)

        bq_sb = persist.tile([128, 2], f32)
        bk_sb = persist.tile([128, 2], f32)
        bv_sb = persist.tile([1, HD], f32r)
        for hdc in range(2):
            nc.sync.dma_start(out=bq_sb[:, hdc : hdc + 1], in_=bq[bass.ts(hdc, 128), :])
            nc.sync.dma_start(out=bk_sb[:, hdc : hdc + 1], in_=bk[bass.ts(hdc, 128), :])
        nc.sync.dma_start(out=bv_sb[:], in_=bv[:])

        ones_f32 = persist.tile([128, 128], f32)
        nc.sync.dma_start(out=ones_f32[:], in_=cones)
        ones_row = persist.tile([1, 128], f32r)
        nc.sync.dma_start(out=ones_row[:], in_=cones[0:1, :].bitcast(f32r))

        # ---- stage W: weights -> wT[c, hd] (generator; interleaves fused(0)) ----
        wqT = persist.tile([128, NCC, HD], f32r)
        wkT = persist.tile([128, NCC, HD], f32r)
        wvT = persist.tile([128, NCC, HD], f32r)

        def gen_w():
            for w_ext, wT in ((wq, wqT), (wk, wkT), (wv, wvT)):
                wn = w_pool.tile([128, 2, D], f32r, tag="wn", name="wn")
                nc.sync.dma_start(
                    out=wn[:], in_=w_ext.rearrange("(c p) d -> p c d", p=128)
                )
                for cc in range(NCC):
                    pw = psum_big.tile([128, 512], f32r, tag="big", name="pw")
                    for hdc in range(2):
                        nc.tensor.transpose(
                            pw[:, bass.ts(hdc, 128)],
                            wn[:, hdc, bass.ts(cc, 128)],
                            ident_r[:],
                        )
                    nc.any.tensor_copy(wT[:, cc, :], pw[:, 0:HD])
                yield

        # ---- fused schedule: per s-group: transpose x + q/k/v proj, then
        # attention for query-group g = sg (causality: needs data <= (sg+1)*512)
        xT = persist.tile([128, NCC, S], f32r)  # 64KB/partition
        qT = persist.tile([128, 2, S], f32r)
        kT = persist.tile([128, 2, S], f32r)
        v_aug = persist.tile([128, NST, NHC, 65], f32r)
        z_full = persist.tile([128, NST, HD], f32)
        nc.scalar.copy(
            v_aug[:, :, :, 64],
            ones_f32[:, 0:64].rearrange("p (a b) -> p a b", a=NST),
        )

        def gen_fused(sg):
            for stl in range(4):
                st = sg * 4 + stl
                xn = x_pool.tile([128, D], f32r, tag="xn", name="xn")
                nc.sync.dma_start(out=xn[:], in_=x[bass.ts(st, 128), :])
                for cb in range(2):
                    pt = psum_big.tile([128, 512], f32r, tag="big", name="pt")
                    for q in range(4):
                        cc = cb * 4 + q
                        nc.tensor.transpose(
                            pt[:, bass.ts(q, 128)], xn[:, bass.ts(cc, 128)], ident_r[:]
                        )
                    nc.any.tensor_copy(
                        xT[:, bass.ts(cb, 4), bass.ts(st, 128)],
                        pt[:].rearrange("p (c s) -> p c s", c=4),
                    )
                yield
            # q/k projections for this 512-wide s-chunk (alternate hdc banks)
            for wT, bias, dstT in ((wqT, bq_sb, qT), (wkT, bk_sb, kT)):
                pa = psum_big.tile([128, 512], f32, tag="big", name="pa")
                pb = psum_big.tile([128, 512], f32, tag="big", name="pb")
                for cc in range(NCC):
                    for hdc, pp in ((0, pa), (1, pb)):
                        nc.tensor.matmul(
                            pp[:],
                            lhsT=wT[:, cc, bass.ts(hdc, 128)],
                            rhs=xT[:, cc, bass.ts(sg, 512)],
                            start=(cc == 0),
                            stop=(cc == NCC - 1),
                        )
                for hdc, pp in ((0, pa), (1, pb)):
                    nc.vector.tensor_scalar_add(
                        dstT[:, hdc, bass.ts(sg, 512)],
                        pp[:],
                        bias[:, hdc : hdc + 1],
                    )
                yield
            # v projection for the 4 s-tiles (pairs, alternating banks)
            for spair in range(2):
                pvs = [
                    psum_big.tile([128, HD], f32, tag="big", name=f"pv{stl}")
                    for stl in range(2)
                ]
                for cc in range(NCC):
                    for stl in range(2):
                        nc.tensor.matmul(
                            pvs[stl][:],
                            lhsT=xT[:, cc, bass.ts(sg * 4 + spair * 2 + stl, 128)],
                            rhs=wvT[:, cc, :],
                            start=(cc == 0),
                            stop=False,
                        )
                for stl in range(2):
                    st = sg * 4 + spair * 2 + stl
                    nc.tensor.matmul(
                        pvs[stl][:],
                        lhsT=ones_row[0:1, :],
                        rhs=bv_sb[0:1, :],
                        start=False,
                        stop=True,
                    )
                    nc.any.tensor_copy(
                        v_aug[:, st, :, 0:64],
                        pvs[stl][:].rearrange("p (h d) -> p h d", h=NHC),
                    )
                yield

        def gen_attn(g, four=False):
            nkc = 4 * g + 4
            nch = NHC if four else 2
            kcb = 1 if four else 2
            for hp in range(0, NHC, nch):
                chains = []
                for ci, h in enumerate(range(hp, hp + nch)):
                    zpool, ztag = (psum_big, "big") if ci >= 2 else (psum_acc, "acc")
                    zp = zpool.tile([65, 512], f32, tag=ztag, name=f"zp{h}")
                    chains.append({"h": h, "zp": zp, "prev": []})

                def emit_scores(ch, kcs):
                    h = ch["h"]
                    po = (h % 2) * 64
                    hdc = h // 2
                    cur = []
                    for kc in kcs:
                        j = kc - 4 * g
                        q0 = max(0, 128 * j)
                        sp = psum_big.tile([128, 512], f32, tag="big", name="sp")
                        nc.tensor.matmul(
                            sp[:, q0:512],
                            lhsT=kT[po : po + 64, hdc, bass.ts(kc, 128)],
                            rhs=qT[po : po + 64, hdc, bass.ds(g * 512 + q0, 512 - q0)],
                            start=True,
                            stop=True,
                        )
                        if j >= 0:
                            nc.vector.tensor_add(
                                sp[:, q0 : q0 + 128], sp[:, q0 : q0 + 128], dmask[:]
                            )
                        u = u_pool.tile([128, 512], f32r, tag="u", name="u")
                        nc.scalar.activation(
                            u[:, q0:512], sp[:, q0:512], AF.Exp, scale=0.125
                        )
                        cur.append((kc, u, q0))
                    return cur

                def flush_pv(ch):
                    for kc, u, q0 in ch["prev"]:
                        nc.tensor.matmul(
                            ch["zp"][:, q0:512],
                            lhsT=v_aug[:, kc, ch["h"], :],
                            rhs=u[:, q0:512],
                            start=(kc == 0),
                            stop=(kc == nkc - 1),
                        )
                    ch["prev"] = []

                for kb in range(0, nkc, kcb):
                    kcs = list(range(kb, min(kb + kcb, nkc)))
                    for ch in chains:
                        cur = emit_scores(ch, kcs)
                        flush_pv(ch)
                        ch["prev"] = cur
                    yield
                for ch in chains:
                    flush_pv(ch)

                for ch in chains:
                    h = ch["h"]
                    zc = zc_pool.tile([65, 512], f32, tag="zc", name="zc")
                    nc.vector.tensor_copy(zc[:], ch["zp"][:])
                    for qt in range(4):
                        zt = psum_acc.tile([128, 65], f32, tag="acc", name="zt")
                        nc.tensor.transpose(
                            zt[:], zc[:, bass.ts(qt, 128)], ident[0:65, 0:65]
                        )
                        r = small.tile([128, 1], f32, tag="r", name="r")
                        nc.vector.reciprocal(r[:], zt[:, 64:65])
                        nc.vector.tensor_scalar_mul(
                            z_full[:, g * 4 + qt, bass.ts(h, 64)], zt[:, 0:64], r[:]
                        )
                    yield
            for qt in range(4):
                st = g * 4 + qt
                nc.sync.dma_start(out=out[bass.ts(st, 128), :], in_=z_full[:, st, :])
            yield

        def drain(gen):
            for _ in gen:
                pass

        # explicit program-order interleave: attention for group g alternates
        # with the fused transpose/projection units of s-group g+1, so every
        # engine queue mixes both work streams; the attn(2) remainder then
        # interleaves with attn(3) (whose accumulators live in the big pool)
        # so the tail runs four chains deep
        f0 = gen_fused(0)
        for _ in gen_w():
            next(f0, None)
        drain(f0)
        for sg in range(NG):
            a = gen_attn(sg)
            f = gen_fused(sg + 1) if sg + 1 < NG else iter(())
            while True:
                sa = next(a, StopIteration)
                sf = next(f, StopIteration)
                if sa is StopIteration and sf is StopIteration:
                    break


def build():
    nc = bacc.Bacc(
        "TRN2", target_bir_lowering=False, debug=False, num_devices=NCORES
    )
    x = nc.dram_tensor("x", [S, D], f32r, kind="ExternalInput")
    wq = nc.dram_tensor("wq", [HD, D], f32r, kind="ExternalInput")
    wk = nc.dram_tensor("wk", [HD, D], f32r, kind="ExternalInput")
    wv = nc.dram_tensor("wv", [HD, D], f32r, kind="ExternalInput")
    bq = nc.dram_tensor("bq", [HD, 1], f32, kind="ExternalInput")
    bk = nc.dram_tensor("bk", [HD, 1], f32, kind="ExternalInput")
    bv = nc.dram_tensor("bv", [1, HD], f32r, kind="ExternalInput")
    cid = nc.dram_tensor("cid", [128, 128], f32r, kind="ExternalInput")
    cmask = nc.dram_tensor("cmask", [128, 128], f32, kind="ExternalInput")
    cones = nc.dram_tensor("cones", [128, 128], f32, kind="ExternalInput")
    out = nc.dram_tensor("out", [S, HD], f32, kind="ExternalOutput")
    with tile.TileContext(nc) as tc:
        _body(
            nc, tc, x.ap(), wq.ap(), wk.ap(), wv.ap(),
            bq.ap(), bk.ap(), bv.ap(), out.ap(),
            cid.ap(), cmask.ap(), cones.ap(),
        )
    nc.compile()
    return nc


_NC_CACHE = None


def _get_nc():
    global _NC_CACHE
    if _NC_CACHE is None:
        _NC_CACHE = build()
    return _NC_CACHE


def make_in_maps(q_input, W_q, b_q, W_k, b_k, W_v, b_v):
    cid = np.eye(128, dtype=np.float32)
    ii = np.arange(128)
    cmask = np.where(ii[None, :] >= ii[:, None], 0.0, -8.0e9).astype(np.float32)
    cones = np.ones((128, 128), dtype=np.float32)
    in_maps = []
    for c in range(NCORES):
        b = c // 4
        hs = slice((c % 4) * HD, (c % 4 + 1) * HD)
        in_maps.append(
            {
                "x": np.ascontiguousarray(q_input[b], dtype=np.float32),
                "wq": np.ascontiguousarray(W_q[hs], dtype=np.float32),
                "wk": np.ascontiguousarray(W_k[hs], dtype=np.float32),
                "wv": np.ascontiguousarray(W_v[hs], dtype=np.float32),
                "bq": np.ascontiguousarray(
                    np.asarray(b_q[hs], dtype=np.float32).reshape(HD, 1)
                ),
                "bk": np.ascontiguousarray(
                    np.asarray(b_k[hs], dtype=np.float32).reshape(HD, 1)
                ),
                "bv": np.ascontiguousarray(
                    np.asarray(b_v[hs], dtype=np.float32).reshape(1, HD)
                ),
                "cid": cid,
                "cmask": cmask,
                "cones": cones,
            }
        )
    return in_maps


def assemble(results):
    full = np.empty((B, S, D), dtype=np.float32)
    for c in range(NCORES):
        b = c // 4
        hs = slice((c % 4) * HD, (c % 4 + 1) * HD)
        full[b, :, hs] = results[c]["out"]
    return full


def _ensure_ntff_hook():
    """Register the axon NTFF profiling hook if the image's antenv lacks it."""
    try:
        from antenv import axon_hooks  # noqa: F401

        return
    except ImportError:
        pass
    import types

    try:
        from trn_agent_boot.trn_boot import _ntff_profile_via_ctypes

        hook = _ntff_profile_via_ctypes("/opt/axon/libaxon_pjrt.so")
    except Exception:
        hook = None
    mod = types.ModuleType("antenv.axon_hooks")
    mod._hook = hook
    mod.get_axon_ntff_profile_hook = lambda: mod._hook

    def _set(h):
        mod._hook = h

    mod.set_axon_ntff_profile_hook = _set
    sys.modules["antenv.axon_hooks"] = mod
    try:
        import antenv

        antenv.axon_hooks = mod
    except ImportError:
        pass


def run(inputs_dict, trace=False):
    """Run on hardware; returns (full_output, BassKernelResults)."""
    nc = _get_nc()
    if trace:
        _ensure_ntff_hook()
        import concourse.bass_utils as _bu

        _bu.upload_artifacts = lambda d: d  # no bucket access in this env
    in_maps = make_in_maps(**{k: np.asarray(v) for k, v in inputs_dict.items()})
    res = run_bass_kernel_spmd(nc, in_maps, core_ids=list(range(NCORES)), trace=trace)
    return assemble(res.results), res


def kernel(**inputs):
    out, _ = run(inputs, trace=False)
    return out

